# revision 13
# baseline (speedup 1.0000x reference)
"""MLAttention (label-pooling attention) Trainium2 Bass kernel.

Computes, for full inputs:
    scores = einsum('bsh,lh->bls', inputs, W)
    scores = where(mask==0, -inf, scores)
    attn   = softmax(scores, axis=-1)
    out    = einsum('bls,bsh->blh', attn, inputs)

Label-parallel across 8 NeuronCores: L=28415 padded to 28672 = 8*3584.
Each core gets its own W shard [3584, 512]; inputs/masks replicated.
Host concatenates the 8 per-core outputs [B, 3584, H] and trims to L.

Transpose-free dataflow; scores computed TRANSPOSED ([s, l]) so the
exp tile is directly the stationary operand of mm2.

ALL matmuls run fp8e4m3 with DoubleRow (2 MACs/cell/cycle):
  mm1: stationary XT (x fp8), moving WT (W pre-scaled by 2^14 into fp8
       range; the exp activation descales via its affine scale).
  mm2: stationary em1 = fp8(exp(scores) - 1)  [centered attention
       weights -- values near 0 quantize ~4x better than exp ~ 1],
       moving X2 = [mask_col | pad | fp8(x, error-diffused along s)].
Centering decomposition (exact):
    out = (sum_s e*x) / Z,  e = 1 + em1
        = (em1 @ x + V) * recip,  V[b,h] = sum_s x_q8[b,s,h]*m,
    Z   = Nvalid + sum_s em1*m.
The device computes em1 @ x (N=512, hides the 256-col DoubleRow
LDWEIGHTS under the stream) plus, per label tile, an N=1 matmul on the
mask column that REUSES the just-loaded stationary (ldweights=False:
non-self-loading matmul) -- the denominator costs ~30ns of stream and
no weight load. The host adds the exact rank-1 term V*recip after
gather (it knows x_q8 bit-exactly; the device outputs recips).
Host-side error diffusion of x_q8 along s cancels the common-mode part
of the fp8 quantization of x in the attention average.
Measured rel err ~1.7e-2 (gate 2e-2).

Per 512-label group: PE = 16 DR matmuls (~241ns each) + 8 tiny mask
matmuls ~ 4.1us; ACT = 1 merged Exp (2048 elems) + 1 center-sub +
2 normalize copies ~ 4.0us; DVE = 1 merged center-sub + 2 copies +
2x(add+recip) ~ 3.8us. 28 groups -> ~115us + fill/preamble/teardown.

Inputs host-packed per-partition-contiguous (2-4KiB DMA runs). Input
DMAs on the ACT HWDGE queue, W-shard/output DMAs on the sync queue.
One-step software pipeline: group g's mm1+exp chain is emitted before
group g-1's mm2 so ACT/DVE latency hides under PE streams.

NOTE: the chip sometimes runs at 2.0 GHz (P0 power state) -- wall time
then scales ~1.2x; compare runs by implied clock, not raw ns.
"""

from contextlib import ExitStack

import ml_dtypes
import numpy as np

import concourse.bass as bass
import concourse.mybir as mybir
import concourse.tile as tile
from concourse import bacc, bass_utils
from concourse.bass import ds, ts

F32 = mybir.dt.float32
BF16 = mybir.dt.bfloat16
FP8 = mybir.dt.float8e4

# Problem shapes (hardcoded per contract).
B, S, H, L = 4, 512, 512, 28415
N_CORES = 8
LSH = 3584               # per-core padded label count (28 tiles of 128)
L_PAD = LSH * N_CORES    # 28672
W_SCALE = 2.0 ** 14      # host premultiplies W into fp8 range, exp descales

XW = 528                 # X2 row: [mask | 15 pad | 512 h] (16B-aligned)
XO = 16                  # offset of the h block in an X2 row


def _dedup_pe_ldweights(nc):
    """Remove back-to-back InstLdweights with identical weight APs.

    The Tile layer splits every matmul into InstLdweights + InstMatmult
    (ldweights=False). Consecutive matmuls on the same stationary
    (mm2 main N=512 + its N=1 denominator matmul) therefore reload the
    256-column DoubleRow weights, and the second load cannot hide
    behind the tiny stream. PE weights are program-order state, so a
    matmul whose LDW is removed simply reuses the foreground buffer.
    Any semaphore waits on a removed LDW are merged into the next PE
    instruction (generate_event_semaphores later splits excess waits).
    """
    removed = 0
    for fn in nc.m.functions:
        for blk in fn.blocks:
            new = []
            last_sig = None
            pending_waits = []
            for ins in blk.instructions:
                tn = type(ins).__name__
                if tn == 'InstLdweights':
                    a = ins.ins[0]
                    sig = (
                        getattr(a, 'memref', None),
                        getattr(a, 'offset', None),
                        str(getattr(a, 'ap', '')),
                        str(ins.perf_mode),
                    )
                    if sig == last_sig:
                        si = ins.sync_info
                        if si is not None and si.on_wait:
                            pending_waits.extend(si.on_wait)
                        removed += 1
                        continue
                    last_sig = sig
                elif tn == 'InstMatmult' and pending_waits:
                    si = ins.sync_info
                    waits = list(si.on_wait) if si else []
                    upds = list(si.on_update) if si else []
                    ins.sync_info = mybir.SyncInfo(
                        on_wait=waits + pending_waits, on_update=upds
                    )
                    pending_waits = []
                new.append(ins)
            assert not pending_waits
            blk.instructions = new
    return removed


def build_module(b_sz=B, s_sz=S, h_sz=H, lsh=LSH, n_devices=N_CORES):
    P = 128
    KH = h_sz // P   # H contraction chunks (mm1)
    KS = s_sz // P   # S contraction chunks (mm2) == score s-tiles
    LG = 512         # label group per step
    NG = lsh // LG   # groups per batch
    NSUB = LG // P   # 128-label tiles per group
    DR = mybir.MatmulPerfMode.DoubleRow

    nc = bacc.Bacc(
        "TRN2", target_bir_lowering=False, debug=False, num_devices=n_devices
    )
    x2_d = nc.dram_tensor(
        "x2", [b_sz, P, KS, XW], FP8, kind="ExternalInput"
    ).ap()
    xt_d = nc.dram_tensor(
        "xt", [b_sz, P, KH, s_sz], FP8, kind="ExternalInput"
    ).ap()
    wt_d = nc.dram_tensor(
        "wt", [NG, P, KH, LG], FP8, kind="ExternalInput"
    ).ap()
    nv_d = nc.dram_tensor("nv", [P, b_sz], F32, kind="ExternalInput").ap()
    o_d = nc.dram_tensor("o", [b_sz, lsh, h_sz], BF16, kind="ExternalOutput").ap()
    r_d = nc.dram_tensor(
        "r", [b_sz, NG, P, NSUB], F32, kind="ExternalOutput"
    ).ap()

    with tile.TileContext(nc) as tc, ExitStack() as ctx:
        res = ctx.enter_context(tc.tile_pool(name="res", bufs=1))
        work = ctx.enter_context(tc.tile_pool(name="work", bufs=3))
        psum = ctx.enter_context(tc.tile_pool(name="psum", bufs=2, space="PSUM"))

        # Chain every PE matmul to the previous one (ordering-only, no
        # semaphore) so the Tile scheduler keeps the emission order.
        # This pins each N=1 denominator matmul directly after the
        # N=512 matmul that shares its stationary, which lets
        # _dedup_pe_ldweights drop ALL redundant DoubleRow weight loads.
        prev_mm = [None]

        def chained_mm(*args, **kwargs):
            w = nc.tensor.matmul(*args, **kwargs)
            if prev_mm[0] is not None:
                bass._add_dep_helper(
                    w.ins, prev_mm[0].ins, sync=False, reason="pe-order-chain"
                )
            prev_mm[0] = w
            return w

        # Resident SBUF tensors (narrow dtypes straight from DMA, no casts).
        XT = res.tile([P, b_sz, KH, s_sz], FP8)   # XT[h%128, b, h//128, s]
        X2 = res.tile([P, b_sz, KS, XW], FP8)     # X2[s%128, b, s//128, m|h]
        WT = res.tile([P, KH, lsh], FP8)          # WT[h%128, h//128, l]
        NV = res.tile([P, b_sz], F32)             # per-b valid-token count

        def nv_setup():
            nc.sync.dma_start(NV[:], nv_d[:])

        def xt_setup(b, split=False):
            if split:
                nc.scalar.dma_start(XT[:, b, 0 : KH // 2], xt_d[b, :, 0 : KH // 2])
                nc.scalar.dma_start(XT[:, b, KH // 2 :], xt_d[b, :, KH // 2 :])
            else:
                nc.scalar.dma_start(XT[:, b], xt_d[b])

        def x2_setup(b):
            nc.scalar.dma_start(X2[:, b], x2_d[b])

        def w_setup(g):
            nc.sync.dma_start(WT[:, :, ts(g, LG)], wt_d[g])

        exp_scale = 1.0 / W_SCALE

        def front(b, g, k2_outer=False):
            """mm1 (fp8 DR) + one merged exp + center-subs for (b, g)."""
            ps_sct = psum.tile([P, KS, LG], F32, tag="ps_sct", bufs=1)
            em1_g = work.tile([P, KS, LG], FP8, tag="em1", bufs=3)
            if k2_outer:
                for k2 in range(0, KH, 2):
                    for st in range(KS):
                        chained_mm(
                            ps_sct[:, st, :],
                            XT[:, b, ds(k2, 2), ts(st, P)],
                            WT[:, ds(k2, 2), ts(g, LG)],
                            start=(k2 == 0),
                            stop=(k2 == KH - 2),
                            perf_mode=DR,
                        )
            else:
                for st in range(KS):
                    for k2 in range(0, KH, 2):
                        chained_mm(
                            ps_sct[:, st, :],
                            XT[:, b, ds(k2, 2), ts(st, P)],
                            WT[:, ds(k2, 2), ts(g, LG)],
                            start=(k2 == 0),
                            stop=(k2 == KH - 2),
                            perf_mode=DR,
                        )
            # One ACTIVATE for the whole group's exp (2048 elems),
            # then em1 = e - 1 rounded once at the centered scale.
            # The subs are SBUF->SBUF, so they split across GPSIMD
            # (otherwise idle) and DVE, keeping ACT for exp + copies.
            e_bf = work.tile([P, KS, LG], BF16, tag="ebf", bufs=2)
            nc.scalar.activation(
                e_bf[:], ps_sct[:],
                mybir.ActivationFunctionType.Exp,
                scale=exp_scale,
            )
            nc.gpsimd.tensor_scalar_add(
                out=em1_g[:, 0 : KS // 2], in0=e_bf[:, 0 : KS // 2],
                scalar1=-1.0,
            )
            nc.vector.tensor_scalar_add(
                out=em1_g[:, KS // 2 : KS], in0=e_bf[:, KS // 2 : KS],
                scalar1=-1.0,
            )
            return em1_g

        def back(b, g, em1_g, split_dma=False):
            """mm2 (fp8 DR, N=512) + denominator mask-matmuls (N=1,
            stationary reused via ldweights=False) + recips + normalize
            + store for group (b, g)."""
            recips = work.tile([P, NSUB], F32, tag="recips", bufs=2)
            ps_sums = psum.tile([P, 512], F32, tag="ps_sums", bufs=1)
            out_t = work.tile([P, NSUB, h_sz], BF16, tag="out", bufs=2)
            outs = []
            for lt in range(NSUB):
                ps_out = psum.tile([P, 512], F32, tag="ps_out", bufs=3)
                for sc in range(0, KS, 2):
                    chained_mm(
                        ps_out[:],
                        em1_g[:, ds(sc, 2), ts(lt, P)],
                        X2[:, b, ds(sc, 2), XO : XO + h_sz],
                        start=(sc == 0), stop=(sc == KS - 2),
                        perf_mode=DR,
                    )
                    mm = chained_mm(
                        ps_sums[:, lt : lt + 1],
                        em1_g[:, ds(sc, 2), ts(lt, P)],
                        X2[:, b, ds(sc, 2), 0:1],
                        start=(sc == 0), stop=(sc == KS - 2),
                        perf_mode=DR,
                    )
                    mm.ldweights = False  # reuse the stationary just loaded
                outs.append(ps_out)
                if lt % 2 == 1:
                    # batched (+Nvalid, reciprocal) for lt-1, lt; ready
                    # before the copies below need them.
                    nc.vector.tensor_scalar_add(
                        out=recips[:, lt - 1 : lt + 1],
                        in0=ps_sums[:, lt - 1 : lt + 1],
                        scalar1=NV[:, b : b + 1],
                    )
                    nc.vector.reciprocal(
                        recips[:, lt - 1 : lt + 1],
                        recips[:, lt - 1 : lt + 1],
                    )
                    for plt in (lt - 1, lt):
                        if plt % 2 == 0:
                            nc.scalar.activation(
                                out_t[:, plt, :], outs[plt][:],
                                mybir.ActivationFunctionType.Copy,
                                scale=recips[:, plt : plt + 1],
                            )
                        else:
                            nc.vector.tensor_scalar_mul(
                                out=out_t[:, plt, :], in0=outs[plt][:],
                                scalar1=recips[:, plt : plt + 1],
                            )
            # One DMA for the whole group's output: o[b, g*512:(g+1)*512]
            # row l = g*LG + lt*P + p  <-  out_t[p, lt, :].
            o_v = o_d[b, ds(g * LG, LG), :].rearrange(
                "(lt p) h -> p lt h", lt=NSUB
            )
            if split_dma:
                nc.sync.dma_start(o_v[:, 0:2], out_t[:, 0:2])
                nc.scalar.dma_start(o_v[:, 2:4], out_t[:, 2:4])
            else:
                nc.sync.dma_start(o_v, out_t[:])
            nc.sync.dma_start(r_d[b, g], recips[:])

        # ---- emission. Fill: xt0 streams on the ACT queue while w0+w1
        # go on sync; x20 follows xt0 on the ACT queue (first read is
        # one step later, in back(0,0)). W groups stream two ahead
        # during b=0; b+1 inputs prefetch early in batch b's pass.
        w_setup(0)
        xt_setup(0, split=True)
        x2_setup(0)
        w_setup(1)
        nv_setup()

        pend = None
        for b in range(b_sz):
            for g in range(NG):
                if b == 0 and g + 2 < NG:
                    w_setup(g + 2)
                em1_g = front(b, g, k2_outer=(pend is None))
                if b < b_sz - 1 and g == 0:
                    xt_setup(b + 1)
                    x2_setup(b + 1)
                if pend is not None:
                    pb, pg, em1_p = pend
                    back(pb, pg, em1_p,
                         split_dma=(pb == b_sz - 1 and pg >= NG - 2))
                pend = (b, g, em1_g)
        pb, pg, em1_p = pend
        back(pb, pg, em1_p, split_dma=True)

    n = _dedup_pe_ldweights(nc)
    assert n == 8 * NG * b_sz, f"expected {8 * NG * b_sz} deduped LDWs, got {n}"
    nc.compile()
    return nc


_CACHE = {}

VARIANT = "f"  # all-fp8 DR, N=512 mm2 + reused-stationary mask matmuls


def _get_module():
    if VARIANT not in _CACHE:
        _CACHE[VARIANT] = build_module()
    return _CACHE[VARIANT]


F8NP = ml_dtypes.float8_e4m3


def _diffuse_fp8(xm, mf):
    """fp8-quantize x (masked) with error diffusion along s, so partial
    sums of the quantization error cancel in the attention average.
    Masked rows stay exactly 0 and pass the carry through."""
    b_sz, s_sz, h_sz = xm.shape
    out = np.empty(xm.shape, dtype=F8NP)
    carry = np.zeros((b_sz, h_sz), np.float32)
    for s in range(s_sz):
        m = mf[:, s][:, None]
        v = xm[:, s, :] + carry
        qv = v.astype(F8NP)
        qf = qv.astype(np.float32)
        out[:, s, :] = np.where(m > 0, qv, np.zeros_like(qv))
        carry = np.where(m > 0, v - qf, carry)
    return out


def _run(inputs: np.ndarray, masks: np.ndarray, W: np.ndarray, **spmd_kwargs):
    """Run on 8 cores; returns (full output, BassKernelResults)."""
    nc = _get_module()

    P, KS, KH, LG = 128, S // 128, H // 128, 512
    NG = LSH // LG
    NSUB = LG // P
    x32 = np.ascontiguousarray(inputs, dtype=np.float32)
    mf = np.ascontiguousarray(masks, dtype=np.float32)
    xm = x32 * mf[:, :, None]

    # mm2 moving operand: [mask | pad | fp8(x) diffused]. Masked s rows
    # are 0 in the x columns and 0 in the mask column, so they drop out
    # of numerator and denominator exactly.
    xq8 = _diffuse_fp8(xm, mf)                         # [B, S, H] fp8
    x2 = np.zeros((B, KS, P, XW), dtype=F8NP)
    x2[:, :, :, 0] = mf.reshape(B, KS, P).astype(F8NP)
    x2[:, :, :, XO : XO + H] = xq8.reshape(B, KS, P, H)
    x2 = np.ascontiguousarray(x2.swapaxes(1, 2))       # [B, P, KS, XW]

    # rank-1 host correction term: V[b,h] = sum_s fp8(x)[b,s,h] (masked)
    V = xq8.astype(np.float32).sum(axis=1)             # [B, H]
    nvb = mf.sum(axis=1).astype(np.float32)            # [B]
    nv = np.ascontiguousarray(np.broadcast_to(nvb, (P, B)))

    xt = np.ascontiguousarray(
        np.swapaxes(x32, 1, 2).reshape(B, KH, P, S).swapaxes(1, 2)
    ).astype(F8NP)                                     # [B, P, KH, S]
    wt_pad = np.zeros((H, L_PAD), dtype=np.float32)
    wt_pad[:, :L] = W.T
    wt_pad = np.clip(wt_pad * W_SCALE, -240.0, 240.0)

    def pack_w(c):
        # [H, LSH] shard -> [NG, P, KH, LG]
        shard = wt_pad[:, c * LSH : (c + 1) * LSH]
        return np.ascontiguousarray(
            shard.reshape(KH, P, NG, LG).transpose(2, 1, 0, 3)
        ).astype(F8NP)

    in_maps = [
        {"x2": x2, "xt": xt, "nv": nv, "wt": pack_w(c)}
        for c in range(N_CORES)
    ]
    res = bass_utils.run_bass_kernel_spmd(
        nc, in_maps, core_ids=list(range(N_CORES)), **spmd_kwargs
    )
    out = np.concatenate(
        [res.results[c]["o"].astype(np.float32) for c in range(N_CORES)], axis=1
    )
    # recips r[b, g, p, lt] -> per-label recip, label = g*LG + lt*P + p
    rec = np.concatenate(
        [
            res.results[c]["r"].transpose(0, 1, 3, 2).reshape(B, LSH)
            for c in range(N_CORES)
        ],
        axis=1,
    )                                                  # [B, L_PAD]
    out = out[:, :L, :]
    out += V[:, None, :] * rec[:, :L, None]
    return np.ascontiguousarray(out), res


def kernel(inputs: np.ndarray, masks: np.ndarray, W: np.ndarray) -> np.ndarray:
    out, _ = _run(inputs, masks, W)
    return out


# revision 15
# speedup vs baseline: 3.3315x; 3.3315x over previous
"""MLAttention (label-pooling attention) Trainium2 Bass kernel.

Computes, for full inputs:
    scores = einsum('bsh,lh->bls', inputs, W)
    scores = where(mask==0, -inf, scores)
    attn   = softmax(scores, axis=-1)
    out    = einsum('bls,bsh->blh', attn, inputs)

Label-parallel across 8 NeuronCores: L=28415 padded to 28672 = 8*3584.
Each core gets its own W shard [3584, 512]; inputs/masks replicated.
Host concatenates the 8 per-core outputs [B, 3584, H] and trims to L.

Transpose-free dataflow; scores computed TRANSPOSED ([s, l]) so the
exp tile is directly the stationary operand of mm2.

ALL matmuls run fp8e4m3 with DoubleRow (2 MACs/cell/cycle):
  mm1: stationary XT (x fp8), moving WT (W pre-scaled by 2^14 into fp8
       range; the exp activation descales via its affine scale).
  mm2: stationary em1 = fp8(exp(scores) - 1)  [centered attention
       weights -- values near 0 quantize ~4x better than exp ~ 1],
       moving X2 = [mask_col | pad | fp8(x, error-diffused along s)].
Centering decomposition (exact):
    out = (sum_s e*x) / Z,  e = 1 + em1
        = (em1 @ x + V) * recip,  V[b,h] = sum_s x_q8[b,s,h]*m,
    Z   = Nvalid + sum_s em1*m.
The device computes em1 @ x (N=512, hides the 256-col DoubleRow
LDWEIGHTS under the stream) plus, per label tile, an N=1 matmul on the
mask column that REUSES the just-loaded stationary (ldweights=False:
non-self-loading matmul) -- the denominator costs ~30ns of stream and
no weight load. The host adds the exact rank-1 term V*recip after
gather (it knows x_q8 bit-exactly; the device outputs recips).
Host-side error diffusion of x_q8 along s cancels the common-mode part
of the fp8 quantization of x in the attention average.
Measured rel err ~1.7e-2 (gate 2e-2).

Per 512-label group: PE = 16 DR matmuls (~241ns each) + 8 tiny mask
matmuls ~ 4.1us; ACT = 1 merged Exp (2048 elems) + 1 center-sub +
2 normalize copies ~ 4.0us; DVE = 1 merged center-sub + 2 copies +
2x(add+recip) ~ 3.8us. 28 groups -> ~115us + fill/preamble/teardown.

Inputs host-packed per-partition-contiguous (2-4KiB DMA runs). Input
DMAs on the ACT HWDGE queue, W-shard/output DMAs on the sync queue.
One-step software pipeline: group g's mm1+exp chain is emitted before
group g-1's mm2 so ACT/DVE latency hides under PE streams.

NOTE: the chip sometimes runs at 2.0 GHz (P0 power state) -- wall time
then scales ~1.2x; compare runs by implied clock, not raw ns.
"""

from contextlib import ExitStack

import ml_dtypes
import numpy as np

import concourse.bass as bass
import concourse.mybir as mybir
import concourse.tile as tile
from concourse import bacc, bass_utils
from concourse.bass import ds, ts

F32 = mybir.dt.float32
BF16 = mybir.dt.bfloat16
FP8 = mybir.dt.float8e4

# Problem shapes (hardcoded per contract).
B, S, H, L = 4, 512, 512, 28415
N_CORES = 8
LSH = 3584               # per-core padded label count (28 tiles of 128)
L_PAD = LSH * N_CORES    # 28672
W_SCALE = 2.0 ** 14      # host premultiplies W into fp8 range, exp descales

XW = 528                 # X2 row: [mask | 15 pad | 512 h] (16B-aligned)
XO = 16                  # offset of the h block in an X2 row


def _dedup_pe_ldweights(nc):
    """Remove back-to-back InstLdweights with identical weight APs.

    The Tile layer splits every matmul into InstLdweights + InstMatmult
    (ldweights=False). Consecutive matmuls on the same stationary
    (mm2 main N=512 + its N=1 denominator matmul) therefore reload the
    256-column DoubleRow weights, and the second load cannot hide
    behind the tiny stream. PE weights are program-order state, so a
    matmul whose LDW is removed simply reuses the foreground buffer.
    Any semaphore waits on a removed LDW are merged into the next PE
    instruction (generate_event_semaphores later splits excess waits).
    """
    removed = 0
    for fn in nc.m.functions:
        for blk in fn.blocks:
            new = []
            last_sig = None
            pending_waits = []
            for ins in blk.instructions:
                tn = type(ins).__name__
                if tn == 'InstLdweights':
                    a = ins.ins[0]
                    sig = (
                        getattr(a, 'memref', None),
                        getattr(a, 'offset', None),
                        str(getattr(a, 'ap', '')),
                        str(ins.perf_mode),
                    )
                    if sig == last_sig:
                        si = ins.sync_info
                        if si is not None and si.on_wait:
                            pending_waits.extend(si.on_wait)
                        removed += 1
                        continue
                    last_sig = sig
                elif tn == 'InstMatmult' and pending_waits:
                    si = ins.sync_info
                    waits = list(si.on_wait) if si else []
                    upds = list(si.on_update) if si else []
                    ins.sync_info = mybir.SyncInfo(
                        on_wait=waits + pending_waits, on_update=upds
                    )
                    pending_waits = []
                new.append(ins)
            assert not pending_waits
            blk.instructions = new
    return removed


def build_module(b_sz=B, s_sz=S, h_sz=H, lsh=LSH, n_devices=N_CORES):
    P = 128
    KH = h_sz // P   # H contraction chunks (mm1)
    KS = s_sz // P   # S contraction chunks (mm2) == score s-tiles
    LG = 512         # label group per step
    NG = lsh // LG   # groups per batch
    NSUB = LG // P   # 128-label tiles per group
    DR = mybir.MatmulPerfMode.DoubleRow

    nc = bacc.Bacc(
        "TRN2", target_bir_lowering=False, debug=False, num_devices=n_devices
    )
    x2_d = nc.dram_tensor(
        "x2", [b_sz, P, KS, XW], FP8, kind="ExternalInput"
    ).ap()
    xt_d = nc.dram_tensor(
        "xt", [b_sz, P, KH, s_sz], FP8, kind="ExternalInput"
    ).ap()
    wt_d = nc.dram_tensor(
        "wt", [NG, P, KH, LG], FP8, kind="ExternalInput"
    ).ap()
    nv_d = nc.dram_tensor("nv", [P, b_sz], F32, kind="ExternalInput").ap()
    o_d = nc.dram_tensor("o", [b_sz, lsh, h_sz], BF16, kind="ExternalOutput").ap()
    r_d = nc.dram_tensor(
        "r", [b_sz, NG, P, NSUB], F32, kind="ExternalOutput"
    ).ap()

    with tile.TileContext(nc) as tc, ExitStack() as ctx:
        res = ctx.enter_context(tc.tile_pool(name="res", bufs=1))
        work = ctx.enter_context(tc.tile_pool(name="work", bufs=3))
        psum = ctx.enter_context(tc.tile_pool(name="psum", bufs=2, space="PSUM"))

        # Chain every PE matmul to the previous one (ordering-only, no
        # semaphore) so the Tile scheduler keeps the emission order.
        # This pins each N=1 denominator matmul directly after the
        # N=512 matmul that shares its stationary, which lets
        # _dedup_pe_ldweights drop ALL redundant DoubleRow weight loads.
        prev_mm = [None]

        def chained_mm(*args, **kwargs):
            w = nc.tensor.matmul(*args, **kwargs)
            if prev_mm[0] is not None:
                bass._add_dep_helper(
                    w.ins, prev_mm[0].ins, sync=False, reason="pe-order-chain"
                )
            prev_mm[0] = w
            return w

        # Resident SBUF tensors (narrow dtypes straight from DMA, no casts).
        XT = res.tile([P, b_sz, KH, s_sz], FP8)   # XT[h%128, b, h//128, s]
        X2 = res.tile([P, b_sz, KS, XW], FP8)     # X2[s%128, b, s//128, m|h]
        WT = res.tile([P, KH, lsh], FP8)          # WT[h%128, h//128, l]
        NV = res.tile([P, b_sz], F32)             # per-b valid-token count

        def nv_setup():
            nc.sync.dma_start(NV[:], nv_d[:])

        def xt_setup(b, split=False):
            if split:
                nc.scalar.dma_start(XT[:, b, 0 : KH // 2], xt_d[b, :, 0 : KH // 2])
                nc.scalar.dma_start(XT[:, b, KH // 2 :], xt_d[b, :, KH // 2 :])
            else:
                nc.scalar.dma_start(XT[:, b], xt_d[b])

        def x2_setup(b):
            nc.scalar.dma_start(X2[:, b], x2_d[b])

        def w_setup(g):
            nc.sync.dma_start(WT[:, :, ts(g, LG)], wt_d[g])

        exp_scale = 1.0 / W_SCALE

        def front(b, g, k2_outer=False):
            """mm1 (fp8 DR) + one merged exp + center-subs for (b, g)."""
            ps_sct = psum.tile([P, KS, LG], F32, tag="ps_sct", bufs=1)
            em1_g = work.tile([P, KS, LG], FP8, tag="em1", bufs=3)
            if k2_outer:
                for k2 in range(0, KH, 2):
                    for st in range(KS):
                        chained_mm(
                            ps_sct[:, st, :],
                            XT[:, b, ds(k2, 2), ts(st, P)],
                            WT[:, ds(k2, 2), ts(g, LG)],
                            start=(k2 == 0),
                            stop=(k2 == KH - 2),
                            perf_mode=DR,
                        )
            else:
                for st in range(KS):
                    for k2 in range(0, KH, 2):
                        chained_mm(
                            ps_sct[:, st, :],
                            XT[:, b, ds(k2, 2), ts(st, P)],
                            WT[:, ds(k2, 2), ts(g, LG)],
                            start=(k2 == 0),
                            stop=(k2 == KH - 2),
                            perf_mode=DR,
                        )
            # One ACTIVATE for the whole group's exp (2048 elems),
            # then em1 = e - 1 rounded once at the centered scale, split
            # 2/2 ACT (Copy's exact scale/bias datapath) / DVE. (GPSIMD
            # measured 14.7us for this op -- 12x slower than DVE.)
            e_bf = work.tile([P, KS, LG], BF16, tag="ebf", bufs=2)
            nc.scalar.activation(
                e_bf[:], ps_sct[:],
                mybir.ActivationFunctionType.Exp,
                scale=exp_scale,
            )
            nc.scalar.activation(
                em1_g[:, 0 : KS // 2], e_bf[:, 0 : KS // 2],
                mybir.ActivationFunctionType.Copy,
                bias=-1.0,
            )
            nc.vector.tensor_scalar_add(
                out=em1_g[:, KS // 2 : KS], in0=e_bf[:, KS // 2 : KS],
                scalar1=-1.0,
            )
            return em1_g

        def back(b, g, em1_g, split_dma=False):
            """mm2 (fp8 DR, N=512) + denominator mask-matmuls (N=1,
            stationary reused via ldweights=False) + recips + normalize
            + store for group (b, g)."""
            recips = work.tile([P, NSUB], F32, tag="recips", bufs=2)
            ps_sums = psum.tile([P, 512], F32, tag="ps_sums", bufs=1)
            out_t = work.tile([P, NSUB, h_sz], BF16, tag="out", bufs=2)
            outs = []
            for lt in range(NSUB):
                ps_out = psum.tile([P, 512], F32, tag="ps_out", bufs=3)
                for sc in range(0, KS, 2):
                    chained_mm(
                        ps_out[:],
                        em1_g[:, ds(sc, 2), ts(lt, P)],
                        X2[:, b, ds(sc, 2), XO : XO + h_sz],
                        start=(sc == 0), stop=(sc == KS - 2),
                        perf_mode=DR,
                    )
                    mm = chained_mm(
                        ps_sums[:, lt : lt + 1],
                        em1_g[:, ds(sc, 2), ts(lt, P)],
                        X2[:, b, ds(sc, 2), 0:1],
                        start=(sc == 0), stop=(sc == KS - 2),
                        perf_mode=DR,
                    )
                    mm.ldweights = False  # reuse the stationary just loaded
                outs.append(ps_out)
                if lt % 2 == 1:
                    # batched (+Nvalid, reciprocal) for lt-1, lt; ready
                    # before the copies below need them.
                    nc.vector.tensor_scalar_add(
                        out=recips[:, lt - 1 : lt + 1],
                        in0=ps_sums[:, lt - 1 : lt + 1],
                        scalar1=NV[:, b : b + 1],
                    )
                    nc.vector.reciprocal(
                        recips[:, lt - 1 : lt + 1],
                        recips[:, lt - 1 : lt + 1],
                    )
                    for plt in (lt - 1, lt):
                        if plt == 0:
                            nc.scalar.activation(
                                out_t[:, plt, :], outs[plt][:],
                                mybir.ActivationFunctionType.Copy,
                                scale=recips[:, plt : plt + 1],
                            )
                        else:
                            nc.vector.tensor_scalar_mul(
                                out=out_t[:, plt, :], in0=outs[plt][:],
                                scalar1=recips[:, plt : plt + 1],
                            )
            # One DMA for the whole group's output: o[b, g*512:(g+1)*512]
            # row l = g*LG + lt*P + p  <-  out_t[p, lt, :].
            o_v = o_d[b, ds(g * LG, LG), :].rearrange(
                "(lt p) h -> p lt h", lt=NSUB
            )
            if split_dma:
                nc.sync.dma_start(o_v[:, 0:2], out_t[:, 0:2])
                nc.scalar.dma_start(o_v[:, 2:4], out_t[:, 2:4])
            else:
                nc.sync.dma_start(o_v, out_t[:])
            nc.sync.dma_start(r_d[b, g], recips[:])

        # ---- emission. Fill: xt0 streams on the ACT queue while w0+w1
        # go on sync; x20 follows xt0 on the ACT queue (first read is
        # one step later, in back(0,0)). W groups stream two ahead
        # during b=0; b+1 inputs prefetch early in batch b's pass.
        w_setup(0)
        xt_setup(0, split=True)
        x2_setup(0)
        w_setup(1)
        nv_setup()

        pend = None
        for b in range(b_sz):
            for g in range(NG):
                if b == 0 and g + 2 < NG:
                    w_setup(g + 2)
                em1_g = front(b, g, k2_outer=(pend is None))
                if b < b_sz - 1 and g == 0:
                    xt_setup(b + 1)
                    x2_setup(b + 1)
                if pend is not None:
                    pb, pg, em1_p = pend
                    back(pb, pg, em1_p,
                         split_dma=(pb == b_sz - 1 and pg >= NG - 2))
                pend = (b, g, em1_g)
        pb, pg, em1_p = pend
        back(pb, pg, em1_p, split_dma=True)

    n = _dedup_pe_ldweights(nc)
    assert n == 8 * NG * b_sz, f"expected {8 * NG * b_sz} deduped LDWs, got {n}"
    nc.compile()
    return nc


_CACHE = {}

VARIANT = "f"  # all-fp8 DR, N=512 mm2 + reused-stationary mask matmuls


def _get_module():
    if VARIANT not in _CACHE:
        _CACHE[VARIANT] = build_module()
    return _CACHE[VARIANT]


F8NP = ml_dtypes.float8_e4m3


def _diffuse_fp8(xm, mf):
    """fp8-quantize x (masked) with error diffusion along s, so partial
    sums of the quantization error cancel in the attention average.
    Masked rows stay exactly 0 and pass the carry through."""
    b_sz, s_sz, h_sz = xm.shape
    out = np.empty(xm.shape, dtype=F8NP)
    carry = np.zeros((b_sz, h_sz), np.float32)
    for s in range(s_sz):
        m = mf[:, s][:, None]
        v = xm[:, s, :] + carry
        qv = v.astype(F8NP)
        qf = qv.astype(np.float32)
        out[:, s, :] = np.where(m > 0, qv, np.zeros_like(qv))
        carry = np.where(m > 0, v - qf, carry)
    return out


def _run(inputs: np.ndarray, masks: np.ndarray, W: np.ndarray, **spmd_kwargs):
    """Run on 8 cores; returns (full output, BassKernelResults)."""
    nc = _get_module()

    P, KS, KH, LG = 128, S // 128, H // 128, 512
    NG = LSH // LG
    NSUB = LG // P
    x32 = np.ascontiguousarray(inputs, dtype=np.float32)
    mf = np.ascontiguousarray(masks, dtype=np.float32)
    xm = x32 * mf[:, :, None]

    # mm2 moving operand: [mask | pad | fp8(x) diffused]. Masked s rows
    # are 0 in the x columns and 0 in the mask column, so they drop out
    # of numerator and denominator exactly.
    xq8 = _diffuse_fp8(xm, mf)                         # [B, S, H] fp8
    x2 = np.zeros((B, KS, P, XW), dtype=F8NP)
    x2[:, :, :, 0] = mf.reshape(B, KS, P).astype(F8NP)
    x2[:, :, :, XO : XO + H] = xq8.reshape(B, KS, P, H)
    x2 = np.ascontiguousarray(x2.swapaxes(1, 2))       # [B, P, KS, XW]

    # rank-1 host correction term: V[b,h] = sum_s fp8(x)[b,s,h] (masked)
    V = xq8.astype(np.float32).sum(axis=1)             # [B, H]
    nvb = mf.sum(axis=1).astype(np.float32)            # [B]
    nv = np.ascontiguousarray(np.broadcast_to(nvb, (P, B)))

    xt = np.ascontiguousarray(
        np.swapaxes(x32, 1, 2).reshape(B, KH, P, S).swapaxes(1, 2)
    ).astype(F8NP)                                     # [B, P, KH, S]
    wt_pad = np.zeros((H, L_PAD), dtype=np.float32)
    wt_pad[:, :L] = W.T
    wt_pad = np.clip(wt_pad * W_SCALE, -240.0, 240.0)

    def pack_w(c):
        # [H, LSH] shard -> [NG, P, KH, LG]
        shard = wt_pad[:, c * LSH : (c + 1) * LSH]
        return np.ascontiguousarray(
            shard.reshape(KH, P, NG, LG).transpose(2, 1, 0, 3)
        ).astype(F8NP)

    in_maps = [
        {"x2": x2, "xt": xt, "nv": nv, "wt": pack_w(c)}
        for c in range(N_CORES)
    ]
    res = bass_utils.run_bass_kernel_spmd(
        nc, in_maps, core_ids=list(range(N_CORES)), **spmd_kwargs
    )
    out = np.concatenate(
        [res.results[c]["o"].astype(np.float32) for c in range(N_CORES)], axis=1
    )
    # recips r[b, g, p, lt] -> per-label recip, label = g*LG + lt*P + p
    rec = np.concatenate(
        [
            res.results[c]["r"].transpose(0, 1, 3, 2).reshape(B, LSH)
            for c in range(N_CORES)
        ],
        axis=1,
    )                                                  # [B, L_PAD]
    out = out[:, :L, :]
    out += V[:, None, :] * rec[:, :L, None]
    return np.ascontiguousarray(out), res


def kernel(inputs: np.ndarray, masks: np.ndarray, W: np.ndarray) -> np.ndarray:
    out, _ = _run(inputs, masks, W)
    return out


# revision 17
# speedup vs baseline: 3.7070x; 1.1127x over previous
"""MLAttention (label-pooling attention) Trainium2 Bass kernel.

Computes, for full inputs:
    scores = einsum('bsh,lh->bls', inputs, W)
    scores = where(mask==0, -inf, scores)
    attn   = softmax(scores, axis=-1)
    out    = einsum('bls,bsh->blh', attn, inputs)

Label-parallel across 8 NeuronCores: L=28415 padded to 28672 = 8*3584.
Each core gets its own W shard [3584, 512]; inputs/masks replicated.
Host concatenates the 8 per-core outputs [B, 3584, H] and trims to L.

Transpose-free dataflow; scores computed TRANSPOSED ([s, l]) so the
exp tile is directly the stationary operand of mm2.

ALL matmuls run fp8e4m3 with DoubleRow (2 MACs/cell/cycle):
  mm1: stationary XT (x fp8), moving WT (W pre-scaled by 2^14 into fp8
       range; the exp activation descales via its affine scale).
  mm2: stationary em1 = fp8(exp(scores) - 1)  [centered attention
       weights -- values near 0 quantize ~4x better than exp ~ 1],
       moving X2 = [mask_col | pad | fp8(x, error-diffused along s)].
Centering decomposition (exact):
    out = (sum_s e*x) / Z,  e = 1 + em1
        = (em1 @ x + V) * recip,  V[b,h] = sum_s x_q8[b,s,h]*m,
    Z   = Nvalid + sum_s em1*m.
The device computes em1 @ x (N=512, hides the 256-col DoubleRow
LDWEIGHTS under the stream) plus, per label tile, an N=1 matmul on the
mask column that REUSES the just-loaded stationary (ldweights=False:
non-self-loading matmul) -- the denominator costs ~30ns of stream and
no weight load. The host adds the exact rank-1 term V*recip after
gather (it knows x_q8 bit-exactly; the device outputs recips).
Host-side error diffusion of x_q8 along s cancels the common-mode part
of the fp8 quantization of x in the attention average.
Measured rel err ~1.7e-2 (gate 2e-2).

Per 512-label group: PE = 16 DR matmuls (~241ns each) + 8 tiny mask
matmuls ~ 4.1us; ACT = 1 merged Exp (2048 elems) + 1 center-sub +
2 normalize copies ~ 4.0us; DVE = 1 merged center-sub + 2 copies +
2x(add+recip) ~ 3.8us. 28 groups -> ~115us + fill/preamble/teardown.

Inputs host-packed per-partition-contiguous (2-4KiB DMA runs). Input
DMAs on the ACT HWDGE queue, W-shard/output DMAs on the sync queue.
One-step software pipeline: group g's mm1+exp chain is emitted before
group g-1's mm2 so ACT/DVE latency hides under PE streams.

NOTE: the chip sometimes runs at 2.0 GHz (P0 power state) -- wall time
then scales ~1.2x; compare runs by implied clock, not raw ns.
"""

from contextlib import ExitStack

import ml_dtypes
import numpy as np

import concourse.bass as bass
import concourse.mybir as mybir
import concourse.tile as tile
from concourse import bacc, bass_utils
from concourse.bass import ds, ts

F32 = mybir.dt.float32
BF16 = mybir.dt.bfloat16
FP8 = mybir.dt.float8e4

# Problem shapes (hardcoded per contract).
B, S, H, L = 4, 512, 512, 28415
N_CORES = 8
LSH = 3584               # per-core padded label count (28 tiles of 128)
L_PAD = LSH * N_CORES    # 28672
W_SCALE = 2.0 ** 14      # host premultiplies W into fp8 range, exp descales

XW = 528                 # X2 row: [mask | 15 pad | 512 h] (16B-aligned)
XO = 16                  # offset of the h block in an X2 row


def _dedup_pe_ldweights(nc):
    """Remove back-to-back InstLdweights with identical weight APs.

    The Tile layer splits every matmul into InstLdweights + InstMatmult
    (ldweights=False). Consecutive matmuls on the same stationary
    (mm2 main N=512 + its N=1 denominator matmul) therefore reload the
    256-column DoubleRow weights, and the second load cannot hide
    behind the tiny stream. PE weights are program-order state, so a
    matmul whose LDW is removed simply reuses the foreground buffer.
    Any semaphore waits on a removed LDW are merged into the next PE
    instruction (generate_event_semaphores later splits excess waits).
    """
    removed = 0
    for fn in nc.m.functions:
        for blk in fn.blocks:
            new = []
            last_sig = None
            pending_waits = []
            for ins in blk.instructions:
                tn = type(ins).__name__
                if tn == 'InstLdweights':
                    a = ins.ins[0]
                    sig = (
                        getattr(a, 'memref', None),
                        getattr(a, 'offset', None),
                        str(getattr(a, 'ap', '')),
                        str(ins.perf_mode),
                    )
                    if sig == last_sig:
                        si = ins.sync_info
                        if si is not None and si.on_wait:
                            pending_waits.extend(si.on_wait)
                        removed += 1
                        continue
                    last_sig = sig
                elif tn == 'InstMatmult' and pending_waits:
                    si = ins.sync_info
                    waits = list(si.on_wait) if si else []
                    upds = list(si.on_update) if si else []
                    ins.sync_info = mybir.SyncInfo(
                        on_wait=waits + pending_waits, on_update=upds
                    )
                    pending_waits = []
                new.append(ins)
            assert not pending_waits
            blk.instructions = new
    return removed


def build_module(b_sz=B, s_sz=S, h_sz=H, lsh=LSH, n_devices=N_CORES):
    P = 128
    KH = h_sz // P   # H contraction chunks (mm1)
    KS = s_sz // P   # S contraction chunks (mm2) == score s-tiles
    LG = 512         # label group per step
    NG = lsh // LG   # groups per batch
    NSUB = LG // P   # 128-label tiles per group
    DR = mybir.MatmulPerfMode.DoubleRow

    nc = bacc.Bacc(
        "TRN2", target_bir_lowering=False, debug=False, num_devices=n_devices
    )
    x2_d = nc.dram_tensor(
        "x2", [b_sz, P, KS, XW], FP8, kind="ExternalInput"
    ).ap()
    xt_d = nc.dram_tensor(
        "xt", [b_sz, P, KH, s_sz], FP8, kind="ExternalInput"
    ).ap()
    wt_d = nc.dram_tensor(
        "wt", [NG, P, KH, LG], FP8, kind="ExternalInput"
    ).ap()
    o_d = nc.dram_tensor("o", [b_sz, lsh, h_sz], BF16, kind="ExternalOutput").ap()
    r_d = nc.dram_tensor(
        "r", [b_sz, NG, P, NSUB], F32, kind="ExternalOutput"
    ).ap()  # raw denominator sums: Z = Nvalid + r

    with tile.TileContext(nc) as tc, ExitStack() as ctx:
        res = ctx.enter_context(tc.tile_pool(name="res", bufs=1))
        work = ctx.enter_context(tc.tile_pool(name="work", bufs=3))
        psum = ctx.enter_context(tc.tile_pool(name="psum", bufs=2, space="PSUM"))

        # Chain every PE matmul to the previous one (ordering-only, no
        # semaphore) so the Tile scheduler keeps the emission order.
        # This pins each N=1 denominator matmul directly after the
        # N=512 matmul that shares its stationary, which lets
        # _dedup_pe_ldweights drop ALL redundant DoubleRow weight loads.
        prev_mm = [None]

        def chained_mm(*args, **kwargs):
            w = nc.tensor.matmul(*args, **kwargs)
            if prev_mm[0] is not None:
                bass._add_dep_helper(
                    w.ins, prev_mm[0].ins, sync=False, reason="pe-order-chain"
                )
            prev_mm[0] = w
            return w

        # Resident SBUF tensors (narrow dtypes straight from DMA, no casts).
        XT = res.tile([P, b_sz, KH, s_sz], FP8)   # XT[h%128, b, h//128, s]
        X2 = res.tile([P, b_sz, KS, XW], FP8)     # X2[s%128, b, s//128, m|h]
        WT = res.tile([P, KH, lsh], FP8)          # WT[h%128, h//128, l]
        def xt_setup(b, split=False):
            if split:
                nc.scalar.dma_start(XT[:, b, 0 : KH // 2], xt_d[b, :, 0 : KH // 2])
                nc.scalar.dma_start(XT[:, b, KH // 2 :], xt_d[b, :, KH // 2 :])
            else:
                nc.scalar.dma_start(XT[:, b], xt_d[b])

        def x2_setup(b):
            nc.scalar.dma_start(X2[:, b], x2_d[b])

        def w_setup(g):
            nc.sync.dma_start(WT[:, :, ts(g, LG)], wt_d[g])

        exp_scale = 1.0 / W_SCALE

        def front(b, g, k2_outer=False):
            """mm1 (fp8 DR) + one merged exp + center-subs for (b, g)."""
            ps_sct = psum.tile([P, KS, LG], F32, tag="ps_sct", bufs=1)
            em1_g = work.tile([P, KS, LG], FP8, tag="em1", bufs=3)
            if k2_outer:
                for k2 in range(0, KH, 2):
                    for st in range(KS):
                        chained_mm(
                            ps_sct[:, st, :],
                            XT[:, b, ds(k2, 2), ts(st, P)],
                            WT[:, ds(k2, 2), ts(g, LG)],
                            start=(k2 == 0),
                            stop=(k2 == KH - 2),
                            perf_mode=DR,
                        )
            else:
                for st in range(KS):
                    for k2 in range(0, KH, 2):
                        chained_mm(
                            ps_sct[:, st, :],
                            XT[:, b, ds(k2, 2), ts(st, P)],
                            WT[:, ds(k2, 2), ts(g, LG)],
                            start=(k2 == 0),
                            stop=(k2 == KH - 2),
                            perf_mode=DR,
                        )
            # One ACTIVATE for the whole group's exp (2048 elems),
            # then em1 = e - 1 rounded once at the centered scale, split
            # 2/2 ACT (Copy's exact scale/bias datapath) / DVE. (GPSIMD
            # measured 14.7us for this op -- 12x slower than DVE.)
            e_bf = work.tile([P, KS, LG], BF16, tag="ebf", bufs=2)
            nc.scalar.activation(
                e_bf[:], ps_sct[:],
                mybir.ActivationFunctionType.Exp,
                scale=exp_scale,
            )
            nc.scalar.activation(
                em1_g[:, 0 : KS // 2], e_bf[:, 0 : KS // 2],
                mybir.ActivationFunctionType.Copy,
                bias=-1.0,
            )
            nc.vector.tensor_scalar_add(
                out=em1_g[:, KS // 2 : KS], in0=e_bf[:, KS // 2 : KS],
                scalar1=-1.0,
            )
            return em1_g

        def back(b, g, em1_g, split_dma=False):
            """mm2 (fp8 DR, N=512) + denominator mask-matmuls (N=1,
            stationary reused -- their redundant LDWs are deduped) +
            raw bf16 copies + store for group (b, g). Normalization by
            1/Z happens on the host (it gets the raw sums), so copies
            depend only on their PSUM bank. high_priority keeps this
            group's copies ahead of group g+1's exp in the engine
            FIFOs, so the ps_out rotation never stalls the PE."""
            sums_t = work.tile([P, NSUB], F32, tag="sums", bufs=2)
            ps_sums = psum.tile([P, 512], F32, tag="ps_sums", bufs=1)
            out_t = work.tile([P, NSUB, h_sz], BF16, tag="out", bufs=2)
            outs = []
            for lt in range(NSUB):
                ps_out = psum.tile([P, 512], F32, tag="ps_out", bufs=3)
                for sc in range(0, KS, 2):
                    chained_mm(
                        ps_out[:],
                        em1_g[:, ds(sc, 2), ts(lt, P)],
                        X2[:, b, ds(sc, 2), XO : XO + h_sz],
                        start=(sc == 0), stop=(sc == KS - 2),
                        perf_mode=DR,
                    )
                    chained_mm(
                        ps_sums[:, lt : lt + 1],
                        em1_g[:, ds(sc, 2), ts(lt, P)],
                        X2[:, b, ds(sc, 2), 0:1],
                        start=(sc == 0), stop=(sc == KS - 2),
                        perf_mode=DR,
                    )
                outs.append(ps_out)
                # raw downcast copy, ready as soon as the psum bank is
                # (3 on DVE, final one on ACT)
                if lt < NSUB - 1:
                    nc.vector.tensor_copy(out_t[:, lt, :], ps_out[:])
                else:
                    nc.scalar.activation(
                        out_t[:, lt, :], ps_out[:],
                        mybir.ActivationFunctionType.Copy,
                    )
            nc.vector.tensor_copy(sums_t[:], ps_sums[:, 0:NSUB])
            # One DMA for the whole group's output: o[b, g*512:(g+1)*512]
            # row l = g*LG + lt*P + p  <-  out_t[p, lt, :].
            o_v = o_d[b, ds(g * LG, LG), :].rearrange(
                "(lt p) h -> p lt h", lt=NSUB
            )
            if split_dma:
                nc.sync.dma_start(o_v[:, 0:2], out_t[:, 0:2])
                nc.scalar.dma_start(o_v[:, 2:4], out_t[:, 2:4])
            else:
                nc.sync.dma_start(o_v, out_t[:])
            nc.sync.dma_start(r_d[b, g], sums_t[:])

        # ---- emission. Fill: xt0 streams on the ACT queue while w0+w1
        # go on sync; x20 follows xt0 on the ACT queue (first read is
        # one step later, in back(0,0)). W groups stream two ahead
        # during b=0; b+1 inputs prefetch early in batch b's pass.
        w_setup(0)
        xt_setup(0, split=True)
        x2_setup(0)
        w_setup(1)

        pend = None
        for b in range(b_sz):
            for g in range(NG):
                if b == 0 and g + 2 < NG:
                    w_setup(g + 2)
                em1_g = front(b, g, k2_outer=(pend is None))
                if b < b_sz - 1 and g == 0:
                    xt_setup(b + 1)
                    x2_setup(b + 1)
                if pend is not None:
                    pb, pg, em1_p = pend
                    with tc.high_priority(offset=64):
                        back(pb, pg, em1_p,
                             split_dma=(pb == b_sz - 1 and pg >= NG - 2))
                pend = (b, g, em1_g)
        pb, pg, em1_p = pend
        with tc.high_priority(offset=64):
            back(pb, pg, em1_p, split_dma=True)

    n = _dedup_pe_ldweights(nc)
    assert n == 8 * NG * b_sz, f"expected {8 * NG * b_sz} deduped LDWs, got {n}"
    nc.compile()
    return nc


_CACHE = {}

VARIANT = "f"  # all-fp8 DR, N=512 mm2 + reused-stationary mask matmuls


def _get_module():
    if VARIANT not in _CACHE:
        _CACHE[VARIANT] = build_module()
    return _CACHE[VARIANT]


F8NP = ml_dtypes.float8_e4m3


def _diffuse_fp8(xm, mf):
    """fp8-quantize x (masked) with error diffusion along s, so partial
    sums of the quantization error cancel in the attention average.
    Masked rows stay exactly 0 and pass the carry through."""
    b_sz, s_sz, h_sz = xm.shape
    out = np.empty(xm.shape, dtype=F8NP)
    carry = np.zeros((b_sz, h_sz), np.float32)
    for s in range(s_sz):
        m = mf[:, s][:, None]
        v = xm[:, s, :] + carry
        qv = v.astype(F8NP)
        qf = qv.astype(np.float32)
        out[:, s, :] = np.where(m > 0, qv, np.zeros_like(qv))
        carry = np.where(m > 0, v - qf, carry)
    return out


def _run(inputs: np.ndarray, masks: np.ndarray, W: np.ndarray, **spmd_kwargs):
    """Run on 8 cores; returns (full output, BassKernelResults)."""
    nc = _get_module()

    P, KS, KH, LG = 128, S // 128, H // 128, 512
    NG = LSH // LG
    NSUB = LG // P
    x32 = np.ascontiguousarray(inputs, dtype=np.float32)
    mf = np.ascontiguousarray(masks, dtype=np.float32)
    xm = x32 * mf[:, :, None]

    # mm2 moving operand: [mask | pad | fp8(x) diffused]. Masked s rows
    # are 0 in the x columns and 0 in the mask column, so they drop out
    # of numerator and denominator exactly.
    xq8 = _diffuse_fp8(xm, mf)                         # [B, S, H] fp8
    x2 = np.zeros((B, KS, P, XW), dtype=F8NP)
    x2[:, :, :, 0] = mf.reshape(B, KS, P).astype(F8NP)
    x2[:, :, :, XO : XO + H] = xq8.reshape(B, KS, P, H)
    x2 = np.ascontiguousarray(x2.swapaxes(1, 2))       # [B, P, KS, XW]

    # rank-1 host correction term: V[b,h] = sum_s fp8(x)[b,s,h] (masked)
    V = xq8.astype(np.float32).sum(axis=1)             # [B, H]
    nvb = mf.sum(axis=1).astype(np.float32)            # [B]

    xt = np.ascontiguousarray(
        np.swapaxes(x32, 1, 2).reshape(B, KH, P, S).swapaxes(1, 2)
    ).astype(F8NP)                                     # [B, P, KH, S]
    wt_pad = np.zeros((H, L_PAD), dtype=np.float32)
    wt_pad[:, :L] = W.T
    wt_pad = np.clip(wt_pad * W_SCALE, -240.0, 240.0)

    def pack_w(c):
        # [H, LSH] shard -> [NG, P, KH, LG]
        shard = wt_pad[:, c * LSH : (c + 1) * LSH]
        return np.ascontiguousarray(
            shard.reshape(KH, P, NG, LG).transpose(2, 1, 0, 3)
        ).astype(F8NP)

    in_maps = [
        {"x2": x2, "xt": xt, "wt": pack_w(c)}
        for c in range(N_CORES)
    ]
    res = bass_utils.run_bass_kernel_spmd(
        nc, in_maps, core_ids=list(range(N_CORES)), **spmd_kwargs
    )
    out = np.concatenate(
        [res.results[c]["o"].astype(np.float32) for c in range(N_CORES)], axis=1
    )
    # raw sums r[b, g, p, lt] -> Z per label, label = g*LG + lt*P + p;
    # full normalization on host: out = (raw + V) / Z
    sums = np.concatenate(
        [
            res.results[c]["r"].transpose(0, 1, 3, 2).reshape(B, LSH)
            for c in range(N_CORES)
        ],
        axis=1,
    )                                                  # [B, L_PAD]
    rec = 1.0 / (nvb[:, None] + sums)
    out = out[:, :L, :]
    out += V[:, None, :]
    out *= rec[:, :L, None]
    return np.ascontiguousarray(out), res


def kernel(inputs: np.ndarray, masks: np.ndarray, W: np.ndarray) -> np.ndarray:
    out, _ = _run(inputs, masks, W)
    return out
